# revision 9
# baseline (speedup 1.0000x reference)
"""Causal self-attention block (qkv_proj + RoPE + SDPA + o_proj) on 8
Trainium2 NeuronCores.

Sharding: DP(batch)=4 x TP(heads)=2. Core c = 2*b + t handles batch b,
heads [8t, 8t+8). Each core computes its batch's qkv projection for its
heads, RoPE, causal attention, and a partial o_proj (Megatron row-shard);
the host sums the two partials per batch.

Matmul operands are bf16 (PE streams 1 cycle/row; fp32 PSUM accumulate).
The softmax-denominator path stays fp32r (12-bit mantissa fp32) so the
normalization is unbiased. End-to-end max rel err vs the fp32 reference
is ~4e-3 (dominated by bf16 operand rounding; fp32 accumulation
everywhere).

Layouts (per core):
  xt [2048 hid, 2048 tok] = X[b]^T host-transposed. Q/K projections emit
  qT/kT [1024 (head,d), 2048 tok] directly (lhsT = W tile, rhs = xt
  tile); RoPE applies in transposed layout (rotate-half = partition
  half-swap via SBUF->SBUF DMA). V emits natural [2048 tok, 1024]
  (lhsT = xt tile, rhs = Wv). Attention per head: scores^T [keys, q] =
  K^T-tile stationary @ q^T moving; exp on ACT (PSUM->SBUF, bf16 out);
  causal masking multiplies a sliding 0/1 window on diagonal blocks;
  fully-masked key blocks are skipped. attn^T [d, q] accumulates V-tile
  @ exp in PSUM. Softmax denominator: fp32r DVE-accumulated exp tiles,
  partition-reduced by a ones-column matmul, broadcast back via an
  outer-product matmul, wide reciprocal_approx_fast, one multiply.
  o_proj: out [tok, 2048] partial = attnT tiles stationary @ o_w rows.
"""
import sys

if "/opt/trn_rl_repo" not in sys.path:
    sys.path.insert(0, "/opt/trn_rl_repo")

import numpy as np

B = 4
S = 2048
H = 2048
NHEADS = 16
HD = 128
ROPE_BASE = 10000.0

TP = 2              # head-parallel ways
NH = NHEADS // TP   # heads per core = 8
DOUT = NH * HD      # q/k/v out dim per core = 1024
NKT = H // 128      # hidden contraction tiles = 16
SCALE = 1.0 / float(np.sqrt(HD))

_CACHE = {}


def _round_fp32r(x: np.ndarray) -> np.ndarray:
    bits = np.ascontiguousarray(x, dtype=np.float32).view(np.uint32)
    rnd = ((bits >> 12) & 1).astype(np.uint32)
    out = (bits + np.uint32(0x7FF) + rnd) & np.uint32(0xFFFFF000)
    return out.view(np.float32)


def _build(has_bias: bool):
    import concourse.tile as tile
    from concourse import bacc, mybir
    from contextlib import ExitStack

    F32 = mybir.dt.float32
    F32R = mybir.dt.float32r
    BF16 = mybir.dt.bfloat16
    EXP = mybir.ActivationFunctionType.Exp

    nc = bacc.Bacc(None)
    xt_d = nc.declare_dram_parameter("xt", [H, S], BF16, isOutput=False)
    wq_d = nc.declare_dram_parameter("wq", [H, DOUT], BF16, isOutput=False)
    wk_d = nc.declare_dram_parameter("wk", [H, DOUT], BF16, isOutput=False)
    wv_d = nc.declare_dram_parameter("wv", [H, DOUT], BF16, isOutput=False)
    ow_d = nc.declare_dram_parameter("ow", [DOUT, H], BF16, isOutput=False)
    cos_d = nc.declare_dram_parameter("cosT", [HD, S], F32, isOutput=False)
    sin_d = nc.declare_dram_parameter("sinTs", [HD, S], F32, isOutput=False)
    msk_d = nc.declare_dram_parameter("maskw", [128, 896], BF16, isOutput=False)
    one_d = nc.declare_dram_parameter("ones", [128, 129], F32R, isOutput=False)
    if has_bias:
        bqkv_d = nc.declare_dram_parameter(
            "bqkv", [1, 3 * DOUT], BF16, isOutput=False
        )
        oneb_d = nc.declare_dram_parameter("onesb", [1, 512], BF16, isOutput=False)
    out_d = nc.declare_dram_parameter("out", [S, H], F32, isOutput=True)

    with ExitStack() as ctx:
        tc = ctx.enter_context(tile.TileContext(nc))
        big = ctx.enter_context(tc.tile_pool(name="big", bufs=2))
        owp = ctx.enter_context(tc.tile_pool(name="owp", bufs=1))
        wst = ctx.enter_context(tc.tile_pool(name="wst", bufs=2))
        wvst = ctx.enter_context(tc.tile_pool(name="wvst", bufs=2))
        cst = ctx.enter_context(tc.tile_pool(name="cst", bufs=1))
        ev = ctx.enter_context(tc.tile_pool(name="ev", bufs=2))
        qst = ctx.enter_context(tc.tile_pool(name="qst", bufs=2))
        est = ctx.enter_context(tc.tile_pool(name="est", bufs=3))
        accp = ctx.enter_context(tc.tile_pool(name="acc", bufs=2))
        fin = ctx.enter_context(tc.tile_pool(name="fin", bufs=2))
        aop = ctx.enter_context(tc.tile_pool(name="ao", bufs=2))
        ap3 = ctx.enter_context(tc.tile_pool(name="ap3", bufs=2))
        op3 = ctx.enter_context(tc.tile_pool(name="op3", bufs=3))
        pmm = ctx.enter_context(tc.tile_pool(name="pmm", bufs=3, space="PSUM"))
        patt = ctx.enter_context(tc.tile_pool(name="patt", bufs=2, space="PSUM"))
        prs = ctx.enter_context(tc.tile_pool(name="prs", bufs=2, space="PSUM"))
        dram = ctx.enter_context(tc.tile_pool(name="dram", bufs=1, space="DRAM"))

        # ---- DRAM scratch (tile-tracked for cross-phase deps) ----
        qT_s = dram.tile([DOUT, S], BF16, tag="qT")
        kT_s = dram.tile([DOUT, S], BF16, tag="kT")
        v_s = dram.tile([S, DOUT], BF16, tag="v")
        # per-q-block attn^T scratch so o_proj(qi) only waits on its block
        aT_qs = [
            dram.tile([DOUT, 512], BF16, tag=f"aT{qi}", name=f"aT{qi}")
            for qi in range(4)
        ]

        # ---- constants ----
        ones_sb = cst.tile([128, 129], F32R, tag="ones")
        nc.sync.dma_start(out=ones_sb, in_=one_d[:, :])
        ones_col = ones_sb[:, 128:129]          # [128, 1] fp32r
        ones_row = ones_sb[0:1, 0:128]          # [1, 128] fp32r
        cos_sb = cst.tile([HD, S], F32, tag="cos")
        nc.sync.dma_start(out=cos_sb, in_=cos_d[:, :])
        sin_sb = cst.tile([HD, S], F32, tag="sin")
        nc.sync.dma_start(out=sin_sb, in_=sin_d[:, :])
        msk_sb = cst.tile([128, 896], BF16, tag="msk")
        nc.sync.dma_start(out=msk_sb, in_=msk_d[:, :])
        if has_bias:
            bqkv_sb = cst.tile([1, 3 * DOUT], BF16, tag="bqkv")
            nc.sync.dma_start(out=bqkv_sb, in_=bqkv_d[:, :])
            onesb_sb = cst.tile([1, 512], BF16, tag="onesb")
            nc.sync.dma_start(out=onesb_sb, in_=oneb_d[:, :])

        # ================= Phase 1: QKV projection =================
        for th in range(2):          # token halves of 1024
            t0 = th * 1024
            xt_t = big.tile([128, NKT, 1024], BF16, tag="big")
            for k in range(NKT):
                nc.sync.dma_start(
                    out=xt_t[:, k, :],
                    in_=xt_d[k * 128:(k + 1) * 128, t0:t0 + 1024],
                )

            # --- Q and K: transposed output + RoPE ---
            for wi, (w_d, out_s) in enumerate(((wq_d, qT_s), (wk_d, kT_s))):
                for dt in range(NH):
                    w_t = wst.tile([128, NKT, 128], BF16, tag="w")
                    nc.sync.dma_start(
                        out=w_t,
                        in_=w_d[:, dt * 128:(dt + 1) * 128].rearrange(
                            "(k p) m -> p k m", p=128
                        ),
                    )
                    for tq in range(2):
                        q0 = t0 + tq * 512
                        ps = pmm.tile([128, 512], F32, tag="pmm")
                        if has_bias:
                            bias = bqkv_sb[:, wi * DOUT + dt * 128:
                                           wi * DOUT + (dt + 1) * 128]
                            nc.tensor.matmul(
                                ps, bias, onesb_sb, start=True, stop=False
                            )
                        for k in range(NKT):
                            nc.tensor.matmul(
                                ps,
                                w_t[:, k, :],
                                xt_t[:, k, tq * 512:(tq + 1) * 512],
                                start=(k == 0 and not has_bias),
                                stop=(k == NKT - 1),
                            )
                        # RoPE: q' = q*cos + halfswap(q)*sin'
                        qs = ev.tile([128, 512], F32, tag="qs")
                        nc.scalar.copy(out=qs, in_=ps)
                        qr = ev.tile([128, 512], F32, tag="qr")
                        nc.sync.dma_start(out=qr[0:64, :], in_=qs[64:128, :])
                        nc.sync.dma_start(out=qr[64:128, :], in_=qs[0:64, :])
                        nc.gpsimd.tensor_mul(qs, qs, cos_sb[:, q0:q0 + 512])
                        nc.gpsimd.tensor_mul(qr, qr, sin_sb[:, q0:q0 + 512])
                        qo = ev.tile([128, 512], BF16, tag="qo")
                        nc.vector.tensor_add(qo, qs, qr)
                        nc.sync.dma_start(
                            out=out_s[dt * 128:(dt + 1) * 128, q0:q0 + 512],
                            in_=qo,
                        )

            # --- V: natural output ---
            for dv in range(2):      # dout chunks of 512
                wv_t = wvst.tile([128, NKT, 512], BF16, tag="wv")
                nc.sync.dma_start(
                    out=wv_t,
                    in_=wv_d[:, dv * 512:(dv + 1) * 512].rearrange(
                        "(k p) m -> p k m", p=128
                    ),
                )
                for tt in range(8):  # token tiles of 128 in this half
                    ps = pmm.tile([128, 512], F32, tag="pmm")
                    if has_bias:
                        bias_v = bqkv_sb[:, 2 * DOUT + dv * 512:
                                         2 * DOUT + (dv + 1) * 512]
                        nc.tensor.matmul(
                            ps, onesb_sb[:, 0:128], bias_v,
                            start=True, stop=False,
                        )
                    for k in range(NKT):
                        nc.tensor.matmul(
                            ps,
                            xt_t[:, k, tt * 128:(tt + 1) * 128],
                            wv_t[:, k, :],
                            start=(k == 0 and not has_bias),
                            stop=(k == NKT - 1),
                        )
                    vo = ev.tile([128, 512], BF16, tag="qo")
                    nc.scalar.copy(out=vo, in_=ps)
                    nc.sync.dma_start(
                        out=v_s[t0 + tt * 128:t0 + (tt + 1) * 128,
                                dv * 512:(dv + 1) * 512],
                        in_=vo,
                    )

        # ===== Phase 2+3: causal attention with interleaved o_proj =====
        # qi outer so o_proj for finished q-blocks (PE-only) overlaps the
        # next q-block's attention (ACT/DVE-bound).
        ow_sb = owp.tile([128, NH, H], BF16, tag="ow")
        nc.sync.dma_start(
            out=ow_sb, in_=ow_d.rearrange("(k p) n -> p k n", p=128)
        )
        for qi in range(4):
            nkj = 4 * (qi + 1)
            for hp in range(NH // 2):    # head pairs
                kv = big.tile([128, 2, 2, NKT, 128], BF16, tag="big")
                for hh in range(2):
                    h = hp * 2 + hh
                    nc.sync.dma_start(
                        out=kv[:, hh, 0, 0:nkj, :],
                        in_=kT_s[h * 128:(h + 1) * 128,
                                 0:128 * nkj].rearrange(
                            "p (kt t) -> p kt t", t=128
                        ),
                    )
                    nc.sync.dma_start(
                        out=kv[:, hh, 1, 0:nkj, :],
                        in_=v_s[0:128 * nkj,
                                h * 128:(h + 1) * 128].rearrange(
                            "(kt p) d -> p kt d", p=128
                        ),
                    )
                for hh in range(2):
                    h = hp * 2 + hh
                    q_sb = qst.tile([128, 512], BF16, tag="q")
                    nc.sync.dma_start(
                        out=q_sb,
                        in_=qT_s[h * 128:(h + 1) * 128,
                                 qi * 512:(qi + 1) * 512],
                    )
                    at_ps = patt.tile([128, 512], F32, tag="patt")
                    ac = accp.tile([128, 512], F32R, tag="acc")
                    for kj in range(nkj):
                        s_ps = pmm.tile([128, 512], F32, tag="pmm")
                        nc.tensor.matmul(
                            s_ps, kv[:, hh, 0, kj, :], q_sb,
                            start=True, stop=True,
                        )
                        e_sb = est.tile([128, 512], BF16, tag="e")
                        nc.scalar.activation(
                            out=e_sb, in_=s_ps, func=EXP, scale=SCALE
                        )
                        m = kj - (nkj - 4)
                        if m >= 0:   # diagonal block: causal window
                            nc.vector.tensor_mul(
                                e_sb, e_sb,
                                msk_sb[:, (3 - m) * 128:(3 - m) * 128 + 512],
                            )
                        nc.tensor.matmul(
                            at_ps, kv[:, hh, 1, kj, :], e_sb,
                            start=(kj == 0), stop=(kj == nkj - 1),
                        )
                        if kj == 0:
                            nc.vector.tensor_copy(ac, e_sb)
                        else:
                            nc.vector.tensor_add(ac, ac, e_sb)
                    # softmax denominator: reduce, broadcast, reciprocal
                    rs_ps = prs.tile([1, 512], F32, tag="prs")
                    nc.tensor.matmul(
                        rs_ps, ones_col, ac, start=True, stop=True
                    )
                    rs_sb = fin.tile([1, 512], F32R, tag="r")
                    nc.scalar.copy(out=rs_sb, in_=rs_ps)
                    bc_ps = pmm.tile([128, 512], F32, tag="pmm")
                    nc.tensor.matmul(
                        bc_ps, ones_row, rs_sb, start=True, stop=True
                    )
                    bc_sb = fin.tile([128, 512], F32, tag="bc")
                    nc.vector.reciprocal_approx_fast(out=bc_sb, in_=bc_ps)
                    ao_sb = aop.tile([128, 512], BF16, tag="ao")
                    nc.vector.tensor_mul(ao_sb, at_ps, bc_sb)
                    nc.sync.dma_start(
                        out=aT_qs[qi][h * 128:(h + 1) * 128, :],
                        in_=ao_sb,
                    )
            # o_proj for this q-block's 4 token tiles (overlaps next qi)
            for tl in range(4):
                tt = qi * 4 + tl
                a_sb = ap3.tile([128, NH, 128], BF16, tag="a")
                nc.sync.dma_start(
                    out=a_sb,
                    in_=aT_qs[qi][:, tl * 128:(tl + 1) * 128].rearrange(
                        "(k p) t -> p k t", p=128
                    ),
                )
                for nh4 in range(4):     # output column blocks of 512
                    ps = pmm.tile([128, 512], F32, tag="pmm")
                    for k in range(NH):
                        nc.tensor.matmul(
                            ps,
                            a_sb[:, k, :],
                            ow_sb[:, k, nh4 * 512:(nh4 + 1) * 512],
                            start=(k == 0),
                            stop=(k == NH - 1),
                        )
                    o_sb = op3.tile([128, 512], F32, tag="o")
                    nc.scalar.copy(out=o_sb, in_=ps)
                    nc.sync.dma_start(
                        out=out_d[tt * 128:(tt + 1) * 128,
                                  nh4 * 512:(nh4 + 1) * 512],
                        in_=o_sb,
                    )

    nc.finalize()
    return nc


def _host_inputs(hidden_states, position_ids, qkv_w, qkv_b, o_w, has_bias):
    """Build the 8 per-core input maps (core c = 2*b + t)."""
    import ml_dtypes

    BF = ml_dtypes.bfloat16
    inv_freq = 1.0 / (
        ROPE_BASE ** (np.arange(0, HD, 2, dtype=np.float32) / HD)
    )
    ones = np.ones((128, 129), np.float32)

    # causal window: maskw[j, x] = 1 if (x - 384) >= j
    xg = np.arange(896)[None, :] - 384
    jg = np.arange(128)[:, None]
    maskw = (xg >= jg).astype(BF)

    per_t = []
    for t in range(TP):
        c0 = t * DOUT
        d = dict(
            wq=qkv_w[:, c0:c0 + DOUT].astype(BF),
            wk=qkv_w[:, H + c0:H + c0 + DOUT].astype(BF),
            wv=qkv_w[:, 2 * H + c0:2 * H + c0 + DOUT].astype(BF),
            ow=o_w[c0:c0 + DOUT, :].astype(BF),
        )
        if has_bias:
            d["bqkv"] = np.concatenate(
                [
                    qkv_b[c0:c0 + DOUT],
                    qkv_b[H + c0:H + c0 + DOUT],
                    qkv_b[2 * H + c0:2 * H + c0 + DOUT],
                ]
            ).reshape(1, 3 * DOUT).astype(BF)
            d["onesb"] = np.ones((1, 512), BF)
        per_t.append(d)

    in_maps = []
    for b in range(B):
        pos = position_ids[b].astype(np.float32)        # [S]
        freqs = pos[None, :] * inv_freq[:, None]        # [HD/2, S]
        emb = np.concatenate([freqs, freqs], axis=0)    # [HD, S]
        cosT = np.cos(emb).astype(np.float32)
        sinT = np.sin(emb).astype(np.float32)
        sinTs = sinT.copy()
        sinTs[0:HD // 2] *= -1.0                        # sign of rotate_half
        xt = np.ascontiguousarray(hidden_states[b].T).astype(BF)
        for t in range(TP):
            in_maps.append(
                dict(
                    xt=xt,
                    cosT=cosT,
                    sinTs=sinTs,
                    maskw=maskw,
                    ones=ones,
                    **per_t[t],
                )
            )
    return in_maps


def _run(in_maps, has_bias, trace=False):
    from concourse.bass_utils import run_bass_kernel_spmd

    key = ("nc", has_bias)
    if key not in _CACHE:
        _CACHE[key] = _build(has_bias)
    return run_bass_kernel_spmd(
        _CACHE[key], in_maps, list(range(2 * B)), trace=trace
    )


def kernel(hidden_states, position_ids, qkv_w, qkv_b, o_w, _trace=False):
    hidden_states = np.asarray(hidden_states, dtype=np.float32)
    position_ids = np.asarray(position_ids)
    qkv_w = np.asarray(qkv_w, dtype=np.float32)
    qkv_b = np.asarray(qkv_b, dtype=np.float32)
    o_w = np.asarray(o_w, dtype=np.float32)

    has_bias = bool(np.any(qkv_b))
    in_maps = _host_inputs(
        hidden_states, position_ids, qkv_w, qkv_b, o_w, has_bias
    )
    res = _run(in_maps, has_bias, trace=_trace)
    _CACHE["last_results"] = res

    out = np.empty((B, S, H), dtype=np.float32)
    for b in range(B):
        out[b] = res.results[2 * b]["out"]
        out[b] += res.results[2 * b + 1]["out"]
    return out


# revision 15
# speedup vs baseline: 1.2183x; 1.2183x over previous
"""Causal self-attention block (qkv_proj + RoPE + SDPA + o_proj) on 8
Trainium2 NeuronCores.

Sharding: DP(batch)=4 x TP(heads)=2. Core c = 2*b + t handles batch b,
heads [8t, 8t+8). Each core computes its batch's qkv projection for its
heads, RoPE, causal attention, and a partial o_proj (Megatron row-shard);
the host sums the two partials per batch.

Matmul operands are bf16 (PE streams 1 cycle/row; fp32 PSUM accumulate).
The softmax-denominator path stays fp32r (12-bit mantissa fp32) so the
normalization is unbiased. End-to-end max rel err vs the fp32 reference
is ~4e-3 (dominated by bf16 operand rounding; fp32 accumulation
everywhere).

Layouts (per core):
  xt [2048 hid, 2048 tok] = X[b]^T host-transposed. Q/K projections emit
  qT/kT [1024 (head,d), 2048 tok] directly (lhsT = W tile, rhs = xt
  tile); RoPE applies in transposed layout (rotate-half = partition
  half-swap via SBUF->SBUF DMA). V emits natural [2048 tok, 1024]
  (lhsT = xt tile, rhs = Wv). Attention per head: scores^T [keys, q] =
  K^T-tile stationary @ q^T moving; exp on ACT (PSUM->SBUF, bf16 out);
  causal masking multiplies a sliding 0/1 window on diagonal blocks;
  fully-masked key blocks are skipped. attn^T [d, q] accumulates V-tile
  @ exp in PSUM. Softmax denominator: fp32r DVE-accumulated exp tiles,
  partition-reduced by a ones-column matmul, broadcast back via an
  outer-product matmul, wide reciprocal_approx_fast, one multiply.
  o_proj: out [tok, 2048] partial = attnT tiles stationary @ o_w rows.
"""
import sys

if "/opt/trn_rl_repo" not in sys.path:
    sys.path.insert(0, "/opt/trn_rl_repo")

import numpy as np

B = 4
S = 2048
H = 2048
NHEADS = 16
HD = 128
ROPE_BASE = 10000.0

TP = 2              # head-parallel ways
NH = NHEADS // TP   # heads per core = 8
DOUT = NH * HD      # q/k/v out dim per core = 1024
NKT = H // 128      # hidden contraction tiles = 16
SCALE = 1.0 / float(np.sqrt(HD))

_CACHE = {}


def _round_fp32r(x: np.ndarray) -> np.ndarray:
    bits = np.ascontiguousarray(x, dtype=np.float32).view(np.uint32)
    rnd = ((bits >> 12) & 1).astype(np.uint32)
    out = (bits + np.uint32(0x7FF) + rnd) & np.uint32(0xFFFFF000)
    return out.view(np.float32)


def _build(has_bias: bool):
    import concourse.tile as tile
    from concourse import bacc, mybir
    from contextlib import ExitStack

    F32 = mybir.dt.float32
    F32R = mybir.dt.float32r
    BF16 = mybir.dt.bfloat16
    EXP = mybir.ActivationFunctionType.Exp

    nc = bacc.Bacc(None)
    xt_d = nc.declare_dram_parameter("xt", [H, S], BF16, isOutput=False)
    wq_d = nc.declare_dram_parameter("wq", [H, DOUT], BF16, isOutput=False)
    wk_d = nc.declare_dram_parameter("wk", [H, DOUT], BF16, isOutput=False)
    wv_d = nc.declare_dram_parameter("wv", [H, DOUT], BF16, isOutput=False)
    ow_d = nc.declare_dram_parameter("ow", [DOUT, H], BF16, isOutput=False)
    cos_d = nc.declare_dram_parameter("cosT", [HD, S], F32, isOutput=False)
    sin_d = nc.declare_dram_parameter("sinTss", [HD, S], F32, isOutput=False)
    msk_d = nc.declare_dram_parameter("maskw", [128, 896], BF16, isOutput=False)
    one_d = nc.declare_dram_parameter("ones", [128, 129], F32R, isOutput=False)
    oneb_col_d = nc.declare_dram_parameter("onescb", [128, 1], BF16, isOutput=False)
    if has_bias:
        bqkv_d = nc.declare_dram_parameter(
            "bqkv", [1, 3 * DOUT], BF16, isOutput=False
        )
        oneb_d = nc.declare_dram_parameter("onesb", [1, 512], BF16, isOutput=False)
    out_d = nc.declare_dram_parameter("out", [S, H], F32, isOutput=True)

    with ExitStack() as ctx:
        tc = ctx.enter_context(tile.TileContext(nc))
        big = ctx.enter_context(tc.tile_pool(name="big", bufs=2))
        owp = ctx.enter_context(tc.tile_pool(name="owp", bufs=1))
        wst = ctx.enter_context(tc.tile_pool(name="wst", bufs=2))
        wvst = ctx.enter_context(tc.tile_pool(name="wvst", bufs=2))
        cst = ctx.enter_context(tc.tile_pool(name="cst", bufs=1))
        ev = ctx.enter_context(tc.tile_pool(name="ev", bufs=2))
        qst = ctx.enter_context(tc.tile_pool(name="qst", bufs=2))
        est = ctx.enter_context(tc.tile_pool(name="est", bufs=3))
        fin = ctx.enter_context(tc.tile_pool(name="fin", bufs=2))
        aop = ctx.enter_context(tc.tile_pool(name="ao", bufs=2))
        ap3 = ctx.enter_context(tc.tile_pool(name="ap3", bufs=2))
        op3 = ctx.enter_context(tc.tile_pool(name="op3", bufs=3))
        pmm = ctx.enter_context(tc.tile_pool(name="pmm", bufs=3, space="PSUM"))
        patt = ctx.enter_context(tc.tile_pool(name="patt", bufs=2, space="PSUM"))
        prs = ctx.enter_context(tc.tile_pool(name="prs", bufs=2, space="PSUM"))
        dram = ctx.enter_context(tc.tile_pool(name="dram", bufs=1, space="DRAM"))

        # ---- DRAM scratch (tile-tracked for cross-phase deps) ----
        qT_s = dram.tile([DOUT, S], BF16, tag="qT")
        kT_s = dram.tile([DOUT, S], BF16, tag="kT")
        v_s = dram.tile([S, DOUT], BF16, tag="v")
        # per-q-block attn^T scratch so o_proj(qi) only waits on its block
        aT_qs = [
            dram.tile([DOUT, 512], BF16, tag=f"aT{qi}", name=f"aT{qi}")
            for qi in range(4)
        ]

        # ---- constants ----
        ones_sb = cst.tile([128, 129], F32R, tag="ones")
        nc.sync.dma_start(out=ones_sb, in_=one_d[:, :])
        ones_row = ones_sb[0:1, 0:128]          # [1, 128] fp32r
        onescb_sb = cst.tile([128, 1], BF16, tag="onescb")
        nc.sync.dma_start(out=onescb_sb, in_=oneb_col_d[:, :])
        cos_sb = cst.tile([HD, S], F32, tag="cos")
        nc.sync.dma_start(out=cos_sb, in_=cos_d[:, :])
        sin_sb = cst.tile([HD, S], F32, tag="sin")
        nc.sync.dma_start(out=sin_sb, in_=sin_d[:, :])
        msk_sb = cst.tile([128, 896], BF16, tag="msk")
        nc.sync.dma_start(out=msk_sb, in_=msk_d[:, :])
        if has_bias:
            bqkv_sb = cst.tile([1, 3 * DOUT], BF16, tag="bqkv")
            nc.sync.dma_start(out=bqkv_sb, in_=bqkv_d[:, :])
            onesb_sb = cst.tile([1, 512], BF16, tag="onesb")
            nc.sync.dma_start(out=onesb_sb, in_=oneb_d[:, :])

        # ================= Phase 1: QKV projection =================
        for th in range(2):          # token halves of 1024
            t0 = th * 1024
            xt_t = big.tile([128, NKT, 1024], BF16, tag="big")
            for k in range(NKT):
                nc.sync.dma_start(
                    out=xt_t[:, k, :],
                    in_=xt_d[k * 128:(k + 1) * 128, t0:t0 + 1024],
                )

            # --- Q and K: transposed output + RoPE ---
            for wi, (w_d, out_s) in enumerate(((wq_d, qT_s), (wk_d, kT_s))):
                for dt in range(NH):
                    w_t = wst.tile([128, NKT, 128], BF16, tag="w")
                    nc.sync.dma_start(
                        out=w_t,
                        in_=w_d[:, dt * 128:(dt + 1) * 128].rearrange(
                            "(k p) m -> p k m", p=128
                        ),
                    )
                    for tq in range(2):
                        q0 = t0 + tq * 512
                        ps = pmm.tile([128, 512], F32, tag="pmm")
                        if has_bias:
                            bias = bqkv_sb[:, wi * DOUT + dt * 128:
                                           wi * DOUT + (dt + 1) * 128]
                            nc.tensor.matmul(
                                ps, bias, onesb_sb, start=True, stop=False
                            )
                        for k in range(NKT):
                            nc.tensor.matmul(
                                ps,
                                w_t[:, k, :],
                                xt_t[:, k, tq * 512:(tq + 1) * 512],
                                start=(k == 0 and not has_bias),
                                stop=(k == NKT - 1),
                            )
                        # RoPE: q' = q*cos + halfswap(q*sin_shuf)
                        # (sinTss is the host-preshuffled, sign-baked sin)
                        qs = ev.tile([128, 512], F32, tag="qs")
                        nc.vector.tensor_mul(qs, ps, cos_sb[:, q0:q0 + 512])
                        t2p = ev.tile([128, 512], F32, tag="t2p")
                        nc.vector.tensor_mul(t2p, ps, sin_sb[:, q0:q0 + 512])
                        qr = ev.tile([128, 512], F32, tag="qr")
                        nc.sync.dma_start(out=qr[0:64, :], in_=t2p[64:128, :])
                        nc.sync.dma_start(out=qr[64:128, :], in_=t2p[0:64, :])
                        qo = ev.tile([128, 512], BF16, tag="qo")
                        nc.vector.tensor_add(qo, qs, qr)
                        nc.sync.dma_start(
                            out=out_s[dt * 128:(dt + 1) * 128, q0:q0 + 512],
                            in_=qo,
                        )

            # --- V: natural output ---
            for dv in range(2):      # dout chunks of 512
                wv_t = wvst.tile([128, NKT, 512], BF16, tag="wv")
                nc.sync.dma_start(
                    out=wv_t,
                    in_=wv_d[:, dv * 512:(dv + 1) * 512].rearrange(
                        "(k p) m -> p k m", p=128
                    ),
                )
                for tt in range(8):  # token tiles of 128 in this half
                    ps = pmm.tile([128, 512], F32, tag="pmm")
                    if has_bias:
                        bias_v = bqkv_sb[:, 2 * DOUT + dv * 512:
                                         2 * DOUT + (dv + 1) * 512]
                        nc.tensor.matmul(
                            ps, onesb_sb[:, 0:128], bias_v,
                            start=True, stop=False,
                        )
                    for k in range(NKT):
                        nc.tensor.matmul(
                            ps,
                            xt_t[:, k, tt * 128:(tt + 1) * 128],
                            wv_t[:, k, :],
                            start=(k == 0 and not has_bias),
                            stop=(k == NKT - 1),
                        )
                    vo = ev.tile([128, 512], BF16, tag="qo")
                    nc.scalar.copy(out=vo, in_=ps)
                    nc.sync.dma_start(
                        out=v_s[t0 + tt * 128:t0 + (tt + 1) * 128,
                                dv * 512:(dv + 1) * 512],
                        in_=vo,
                    )

        # ================= Phase 2: causal attention =================
        ow_sb = owp.tile([128, NH, H], BF16, tag="ow")
        nc.sync.dma_start(
            out=ow_sb, in_=ow_d.rearrange("(k p) n -> p k n", p=128)
        )
        for hp in range(NH // 2):    # head pairs
            kv = big.tile([128, 2, 2, NKT, 128], BF16, tag="big")
            for hh in range(2):
                h = hp * 2 + hh
                nc.sync.dma_start(
                    out=kv[:, hh, 0, :, :],
                    in_=kT_s[h * 128:(h + 1) * 128, :].rearrange(
                        "p (kt t) -> p kt t", t=128
                    ),
                )
                nc.sync.dma_start(
                    out=kv[:, hh, 1, :, :],
                    in_=v_s[:, h * 128:(h + 1) * 128].rearrange(
                        "(kt p) d -> p kt d", p=128
                    ),
                )
            for hh in range(2):
                h = hp * 2 + hh
                q_sb = qst.tile([128, S], BF16, tag="q")
                nc.sync.dma_start(
                    out=q_sb, in_=qT_s[h * 128:(h + 1) * 128, :]
                )
                for qi in range(4):
                    nkj = 4 * (qi + 1)
                    qsl = q_sb[:, qi * 512:(qi + 1) * 512]
                    at_ps = patt.tile([128, 512], F32, tag="patt")
                    rs_ps = prs.tile([1, 512], F32, tag="prs")
                    for kj in range(nkj):
                        s_ps = pmm.tile([128, 512], F32, tag="pmm")
                        nc.tensor.matmul(
                            s_ps, kv[:, hh, 0, kj, :], qsl,
                            start=True, stop=True,
                        )
                        e_sb = est.tile([128, 512], BF16, tag="e")
                        nc.scalar.activation(
                            out=e_sb, in_=s_ps, func=EXP, scale=SCALE
                        )
                        m = kj - (nkj - 4)
                        if m >= 0:   # diagonal block: causal window
                            nc.vector.tensor_mul(
                                e_sb, e_sb,
                                msk_sb[:, (3 - m) * 128:(3 - m) * 128 + 512],
                            )
                        nc.tensor.matmul(
                            at_ps, kv[:, hh, 1, kj, :], e_sb,
                            start=(kj == 0), stop=(kj == nkj - 1),
                        )
                        # softmax denominator accumulates on the PE
                        nc.tensor.matmul(
                            rs_ps, onescb_sb, e_sb,
                            start=(kj == 0), stop=(kj == nkj - 1),
                        )
                    # broadcast denominator, reciprocal, normalize
                    rs_sb = fin.tile([1, 512], F32R, tag="r")
                    nc.scalar.copy(out=rs_sb, in_=rs_ps)
                    bc_ps = pmm.tile([128, 512], F32, tag="pmm")
                    nc.tensor.matmul(
                        bc_ps, ones_row, rs_sb, start=True, stop=True
                    )
                    bc_sb = fin.tile([128, 512], F32, tag="bc")
                    nc.vector.reciprocal_approx_fast(out=bc_sb, in_=bc_ps)
                    ao_sb = aop.tile([128, 512], BF16, tag="ao")
                    nc.vector.tensor_mul(ao_sb, at_ps, bc_sb)
                    nc.sync.dma_start(
                        out=aT_qs[qi][h * 128:(h + 1) * 128, :],
                        in_=ao_sb,
                    )

        # ================= Phase 3: o_proj (partial) =================
        for qi in range(4):
            for tl in range(4):
                tt = qi * 4 + tl
                a_sb = ap3.tile([128, NH, 128], BF16, tag="a")
                nc.sync.dma_start(
                    out=a_sb,
                    in_=aT_qs[qi][:, tl * 128:(tl + 1) * 128].rearrange(
                        "(k p) t -> p k t", p=128
                    ),
                )
                for nh4 in range(4):     # output column blocks of 512
                    ps = pmm.tile([128, 512], F32, tag="pmm")
                    for k in range(NH):
                        nc.tensor.matmul(
                            ps,
                            a_sb[:, k, :],
                            ow_sb[:, k, nh4 * 512:(nh4 + 1) * 512],
                            start=(k == 0),
                            stop=(k == NH - 1),
                        )
                    o_sb = op3.tile([128, 512], F32, tag="o")
                    nc.scalar.copy(out=o_sb, in_=ps)
                    nc.sync.dma_start(
                        out=out_d[tt * 128:(tt + 1) * 128,
                                  nh4 * 512:(nh4 + 1) * 512],
                        in_=o_sb,
                    )

    nc.finalize()
    return nc


def _host_inputs(hidden_states, position_ids, qkv_w, qkv_b, o_w, has_bias):
    """Build the 8 per-core input maps (core c = 2*b + t)."""
    import ml_dtypes

    BF = ml_dtypes.bfloat16
    inv_freq = 1.0 / (
        ROPE_BASE ** (np.arange(0, HD, 2, dtype=np.float32) / HD)
    )
    ones = np.ones((128, 129), np.float32)

    # causal window: maskw[j, x] = 1 if (x - 384) >= j
    xg = np.arange(896)[None, :] - 384
    jg = np.arange(128)[:, None]
    maskw = (xg >= jg).astype(BF)

    per_t = []
    for t in range(TP):
        c0 = t * DOUT
        d = dict(
            wq=qkv_w[:, c0:c0 + DOUT].astype(BF),
            wk=qkv_w[:, H + c0:H + c0 + DOUT].astype(BF),
            wv=qkv_w[:, 2 * H + c0:2 * H + c0 + DOUT].astype(BF),
            ow=o_w[c0:c0 + DOUT, :].astype(BF),
        )
        if has_bias:
            d["bqkv"] = np.concatenate(
                [
                    qkv_b[c0:c0 + DOUT],
                    qkv_b[H + c0:H + c0 + DOUT],
                    qkv_b[2 * H + c0:2 * H + c0 + DOUT],
                ]
            ).reshape(1, 3 * DOUT).astype(BF)
            d["onesb"] = np.ones((1, 512), BF)
        per_t.append(d)

    in_maps = []
    for b in range(B):
        pos = position_ids[b].astype(np.float32)        # [S]
        freqs = pos[None, :] * inv_freq[:, None]        # [HD/2, S]
        emb = np.concatenate([freqs, freqs], axis=0)    # [HD, S]
        cosT = np.cos(emb).astype(np.float32)
        sinT = np.sin(emb).astype(np.float32)
        # pre-shuffled, sign-baked sin: t2_pre = q * sinTss, then swapping
        # partition halves of t2_pre yields rotate_half(q)*sin.
        sinTss = np.empty_like(sinT)
        sinTss[0:HD // 2] = sinT[HD // 2:HD]            # rows d<64: +sin[d+64]
        sinTss[HD // 2:HD] = -sinT[0:HD // 2]           # rows d>=64: -sin[d-64]
        xt = np.ascontiguousarray(hidden_states[b].T).astype(BF)
        for t in range(TP):
            in_maps.append(
                dict(
                    xt=xt,
                    cosT=cosT,
                    sinTss=sinTss,
                    maskw=maskw,
                    ones=ones,
                    onescb=np.ones((128, 1), BF),
                    **per_t[t],
                )
            )
    return in_maps


def _run(in_maps, has_bias, trace=False):
    from concourse.bass_utils import run_bass_kernel_spmd

    key = ("nc", has_bias)
    if key not in _CACHE:
        _CACHE[key] = _build(has_bias)
    return run_bass_kernel_spmd(
        _CACHE[key], in_maps, list(range(2 * B)), trace=trace
    )


def kernel(hidden_states, position_ids, qkv_w, qkv_b, o_w, _trace=False):
    hidden_states = np.asarray(hidden_states, dtype=np.float32)
    position_ids = np.asarray(position_ids)
    qkv_w = np.asarray(qkv_w, dtype=np.float32)
    qkv_b = np.asarray(qkv_b, dtype=np.float32)
    o_w = np.asarray(o_w, dtype=np.float32)

    has_bias = bool(np.any(qkv_b))
    in_maps = _host_inputs(
        hidden_states, position_ids, qkv_w, qkv_b, o_w, has_bias
    )
    res = _run(in_maps, has_bias, trace=_trace)
    _CACHE["last_results"] = res

    out = np.empty((B, S, H), dtype=np.float32)
    for b in range(B):
        out[b] = res.results[2 * b]["out"]
        out[b] += res.results[2 * b + 1]["out"]
    return out


# revision 16
# speedup vs baseline: 1.2430x; 1.0203x over previous
"""Causal self-attention block (qkv_proj + RoPE + SDPA + o_proj) on 8
Trainium2 NeuronCores.

Sharding: DP(batch)=4 x TP(heads)=2. Core c = 2*b + t handles batch b,
heads [8t, 8t+8). Each core computes its batch's qkv projection for its
heads, RoPE, causal attention, and a partial o_proj (Megatron row-shard);
the host sums the two partials per batch.

Matmul operands are bf16 (PE streams 1 cycle/row; fp32 PSUM accumulate).
The softmax-denominator path stays fp32r (12-bit mantissa fp32) so the
normalization is unbiased. End-to-end max rel err vs the fp32 reference
is ~4e-3 (dominated by bf16 operand rounding; fp32 accumulation
everywhere).

Layouts (per core):
  xt [2048 hid, 2048 tok] = X[b]^T host-transposed. Q/K projections emit
  qT/kT [1024 (head,d), 2048 tok] directly (lhsT = W tile, rhs = xt
  tile); RoPE applies in transposed layout (rotate-half = partition
  half-swap via SBUF->SBUF DMA). V emits natural [2048 tok, 1024]
  (lhsT = xt tile, rhs = Wv). Attention per head: scores^T [keys, q] =
  K^T-tile stationary @ q^T moving; exp on ACT (PSUM->SBUF, bf16 out);
  causal masking multiplies a sliding 0/1 window on diagonal blocks;
  fully-masked key blocks are skipped. attn^T [d, q] accumulates V-tile
  @ exp in PSUM. Softmax denominator: fp32r DVE-accumulated exp tiles,
  partition-reduced by a ones-column matmul, broadcast back via an
  outer-product matmul, wide reciprocal_approx_fast, one multiply.
  o_proj: out [tok, 2048] partial = attnT tiles stationary @ o_w rows.
"""
import sys

if "/opt/trn_rl_repo" not in sys.path:
    sys.path.insert(0, "/opt/trn_rl_repo")

import numpy as np

B = 4
S = 2048
H = 2048
NHEADS = 16
HD = 128
ROPE_BASE = 10000.0

TP = 2              # head-parallel ways
NH = NHEADS // TP   # heads per core = 8
DOUT = NH * HD      # q/k/v out dim per core = 1024
NKT = H // 128      # hidden contraction tiles = 16
SCALE = 1.0 / float(np.sqrt(HD))

_CACHE = {}


def _round_fp32r(x: np.ndarray) -> np.ndarray:
    bits = np.ascontiguousarray(x, dtype=np.float32).view(np.uint32)
    rnd = ((bits >> 12) & 1).astype(np.uint32)
    out = (bits + np.uint32(0x7FF) + rnd) & np.uint32(0xFFFFF000)
    return out.view(np.float32)


def _build(has_bias: bool):
    import concourse.tile as tile
    from concourse import bacc, mybir
    from contextlib import ExitStack

    F32 = mybir.dt.float32
    F32R = mybir.dt.float32r
    BF16 = mybir.dt.bfloat16
    EXP = mybir.ActivationFunctionType.Exp

    nc = bacc.Bacc(None)
    xt_d = nc.declare_dram_parameter("xt", [H, S], BF16, isOutput=False)
    wq_d = nc.declare_dram_parameter("wq", [H, DOUT], BF16, isOutput=False)
    wk_d = nc.declare_dram_parameter("wk", [H, DOUT], BF16, isOutput=False)
    wv_d = nc.declare_dram_parameter("wv", [H, DOUT], BF16, isOutput=False)
    ow_d = nc.declare_dram_parameter("ow", [DOUT, H], BF16, isOutput=False)
    cos_d = nc.declare_dram_parameter("cosT", [HD, S], F32, isOutput=False)
    sin_d = nc.declare_dram_parameter("sinTss", [HD, S], F32, isOutput=False)
    msk_d = nc.declare_dram_parameter("maskw", [128, 896], BF16, isOutput=False)
    one_d = nc.declare_dram_parameter("ones", [128, 129], F32R, isOutput=False)
    oneb_col_d = nc.declare_dram_parameter("onescb", [128, 1], BF16, isOutput=False)
    if has_bias:
        bqkv_d = nc.declare_dram_parameter(
            "bqkv", [1, 3 * DOUT], BF16, isOutput=False
        )
        oneb_d = nc.declare_dram_parameter("onesb", [1, 512], BF16, isOutput=False)
    out_d = nc.declare_dram_parameter("out", [S, H], F32, isOutput=True)

    with ExitStack() as ctx:
        tc = ctx.enter_context(tile.TileContext(nc))
        big = ctx.enter_context(tc.tile_pool(name="big", bufs=2))
        owp = ctx.enter_context(tc.tile_pool(name="owp", bufs=1))
        wst = ctx.enter_context(tc.tile_pool(name="wst", bufs=2))
        wvst = ctx.enter_context(tc.tile_pool(name="wvst", bufs=2))
        cst = ctx.enter_context(tc.tile_pool(name="cst", bufs=1))
        ev = ctx.enter_context(tc.tile_pool(name="ev", bufs=2))
        qst = ctx.enter_context(tc.tile_pool(name="qst", bufs=2))
        est = ctx.enter_context(tc.tile_pool(name="est", bufs=3))
        fin = ctx.enter_context(tc.tile_pool(name="fin", bufs=2))
        aop = ctx.enter_context(tc.tile_pool(name="ao", bufs=2))
        ap3 = ctx.enter_context(tc.tile_pool(name="ap3", bufs=2))
        op3 = ctx.enter_context(tc.tile_pool(name="op3", bufs=3))
        pmm = ctx.enter_context(tc.tile_pool(name="pmm", bufs=4, space="PSUM"))
        patt = ctx.enter_context(tc.tile_pool(name="patt", bufs=2, space="PSUM"))
        prs = ctx.enter_context(tc.tile_pool(name="prs", bufs=2, space="PSUM"))
        dram = ctx.enter_context(tc.tile_pool(name="dram", bufs=1, space="DRAM"))

        # ---- DRAM scratch (tile-tracked for cross-phase deps) ----
        qT_s = dram.tile([DOUT, S], BF16, tag="qT")
        kT_s = dram.tile([DOUT, S], BF16, tag="kT")
        v_s = dram.tile([S, DOUT], BF16, tag="v")
        # per-q-block attn^T scratch so o_proj(qi) only waits on its block
        aT_qs = [
            dram.tile([DOUT, 512], BF16, tag=f"aT{qi}", name=f"aT{qi}")
            for qi in range(4)
        ]

        # ---- constants ----
        ones_sb = cst.tile([128, 129], F32R, tag="ones")
        nc.sync.dma_start(out=ones_sb, in_=one_d[:, :])
        ones_row = ones_sb[0:1, 0:128]          # [1, 128] fp32r
        onescb_sb = cst.tile([128, 1], BF16, tag="onescb")
        nc.sync.dma_start(out=onescb_sb, in_=oneb_col_d[:, :])
        cos_sb = cst.tile([HD, S], F32, tag="cos")
        nc.sync.dma_start(out=cos_sb, in_=cos_d[:, :])
        sin_sb = cst.tile([HD, S], F32, tag="sin")
        nc.sync.dma_start(out=sin_sb, in_=sin_d[:, :])
        msk_sb = cst.tile([128, 896], BF16, tag="msk")
        nc.sync.dma_start(out=msk_sb, in_=msk_d[:, :])
        if has_bias:
            bqkv_sb = cst.tile([1, 3 * DOUT], BF16, tag="bqkv")
            nc.sync.dma_start(out=bqkv_sb, in_=bqkv_d[:, :])
            onesb_sb = cst.tile([1, 512], BF16, tag="onesb")
            nc.sync.dma_start(out=onesb_sb, in_=oneb_d[:, :])

        # ================= Phase 1: QKV projection =================
        for th in range(2):          # token halves of 1024
            t0 = th * 1024
            xt_t = big.tile([128, NKT, 1024], BF16, tag="big")
            for k in range(NKT):
                nc.sync.dma_start(
                    out=xt_t[:, k, :],
                    in_=xt_d[k * 128:(k + 1) * 128, t0:t0 + 1024],
                )

            # --- Q and K: transposed output + RoPE ---
            for wi, (w_d, out_s) in enumerate(((wq_d, qT_s), (wk_d, kT_s))):
                for dt in range(NH):
                    w_t = wst.tile([128, NKT, 128], BF16, tag="w")
                    nc.sync.dma_start(
                        out=w_t,
                        in_=w_d[:, dt * 128:(dt + 1) * 128].rearrange(
                            "(k p) m -> p k m", p=128
                        ),
                    )
                    for tq in range(2):
                        q0 = t0 + tq * 512
                        ps = pmm.tile([128, 512], F32, tag="pmm")
                        if has_bias:
                            bias = bqkv_sb[:, wi * DOUT + dt * 128:
                                           wi * DOUT + (dt + 1) * 128]
                            nc.tensor.matmul(
                                ps, bias, onesb_sb, start=True, stop=False
                            )
                        for k in range(NKT):
                            nc.tensor.matmul(
                                ps,
                                w_t[:, k, :],
                                xt_t[:, k, tq * 512:(tq + 1) * 512],
                                start=(k == 0 and not has_bias),
                                stop=(k == NKT - 1),
                            )
                        # RoPE: q' = q*cos + halfswap(q*sin_shuf)
                        # (sinTss is the host-preshuffled, sign-baked sin)
                        qs = ev.tile([128, 512], F32, tag="qs")
                        nc.vector.tensor_mul(qs, ps, cos_sb[:, q0:q0 + 512])
                        t2p = ev.tile([128, 512], F32, tag="t2p")
                        nc.vector.tensor_mul(t2p, ps, sin_sb[:, q0:q0 + 512])
                        qr = ev.tile([128, 512], F32, tag="qr")
                        nc.sync.dma_start(out=qr[0:64, :], in_=t2p[64:128, :])
                        nc.sync.dma_start(out=qr[64:128, :], in_=t2p[0:64, :])
                        qo = ev.tile([128, 512], BF16, tag="qo")
                        nc.vector.tensor_add(qo, qs, qr)
                        nc.sync.dma_start(
                            out=out_s[dt * 128:(dt + 1) * 128, q0:q0 + 512],
                            in_=qo,
                        )

            # --- V: natural output ---
            for dv in range(2):      # dout chunks of 512
                wv_t = wvst.tile([128, NKT, 512], BF16, tag="wv")
                nc.sync.dma_start(
                    out=wv_t,
                    in_=wv_d[:, dv * 512:(dv + 1) * 512].rearrange(
                        "(k p) m -> p k m", p=128
                    ),
                )
                for tt in range(8):  # token tiles of 128 in this half
                    ps = pmm.tile([128, 512], F32, tag="pmm")
                    if has_bias:
                        bias_v = bqkv_sb[:, 2 * DOUT + dv * 512:
                                         2 * DOUT + (dv + 1) * 512]
                        nc.tensor.matmul(
                            ps, onesb_sb[:, 0:128], bias_v,
                            start=True, stop=False,
                        )
                    for k in range(NKT):
                        nc.tensor.matmul(
                            ps,
                            xt_t[:, k, tt * 128:(tt + 1) * 128],
                            wv_t[:, k, :],
                            start=(k == 0 and not has_bias),
                            stop=(k == NKT - 1),
                        )
                    vo = ev.tile([128, 512], BF16, tag="qo")
                    nc.scalar.copy(out=vo, in_=ps)
                    nc.sync.dma_start(
                        out=v_s[t0 + tt * 128:t0 + (tt + 1) * 128,
                                dv * 512:(dv + 1) * 512],
                        in_=vo,
                    )

        # ================= Phase 2: causal attention =================
        ow_sb = owp.tile([128, NH, H], BF16, tag="ow")
        nc.sync.dma_start(
            out=ow_sb, in_=ow_d.rearrange("(k p) n -> p k n", p=128)
        )
        for hp in range(NH // 2):    # head pairs
            kv = big.tile([128, 2, 2, NKT, 128], BF16, tag="big")
            for hh in range(2):
                h = hp * 2 + hh
                nc.sync.dma_start(
                    out=kv[:, hh, 0, :, :],
                    in_=kT_s[h * 128:(h + 1) * 128, :].rearrange(
                        "p (kt t) -> p kt t", t=128
                    ),
                )
                nc.sync.dma_start(
                    out=kv[:, hh, 1, :, :],
                    in_=v_s[:, h * 128:(h + 1) * 128].rearrange(
                        "(kt p) d -> p kt d", p=128
                    ),
                )
            for hh in range(2):
                h = hp * 2 + hh
                q_sb = qst.tile([128, S], BF16, tag="q")
                nc.sync.dma_start(
                    out=q_sb, in_=qT_s[h * 128:(h + 1) * 128, :]
                )
                for qi in (3, 2, 1, 0):
                    nkj = 4 * (qi + 1)
                    qsl = q_sb[:, qi * 512:(qi + 1) * 512]
                    at_ps = patt.tile([128, 512], F32, tag="patt")
                    rs_ps = prs.tile([1, 512], F32, tag="prs")
                    for kj in range(nkj):
                        s_ps = pmm.tile([128, 512], F32, tag="pmm")
                        nc.tensor.matmul(
                            s_ps, kv[:, hh, 0, kj, :], qsl,
                            start=True, stop=True,
                        )
                        e_sb = est.tile([128, 512], BF16, tag="e")
                        nc.scalar.activation(
                            out=e_sb, in_=s_ps, func=EXP, scale=SCALE
                        )
                        m = kj - (nkj - 4)
                        if m >= 0:   # diagonal block: causal window
                            nc.vector.tensor_mul(
                                e_sb, e_sb,
                                msk_sb[:, (3 - m) * 128:(3 - m) * 128 + 512],
                            )
                        nc.tensor.matmul(
                            at_ps, kv[:, hh, 1, kj, :], e_sb,
                            start=(kj == 0), stop=(kj == nkj - 1),
                        )
                        # softmax denominator accumulates on the PE
                        nc.tensor.matmul(
                            rs_ps, onescb_sb, e_sb,
                            start=(kj == 0), stop=(kj == nkj - 1),
                        )
                    # broadcast denominator, reciprocal, normalize
                    rs_sb = fin.tile([1, 512], F32R, tag="r")
                    nc.scalar.copy(out=rs_sb, in_=rs_ps)
                    bc_ps = pmm.tile([128, 512], F32, tag="pmm")
                    nc.tensor.matmul(
                        bc_ps, ones_row, rs_sb, start=True, stop=True
                    )
                    bc_sb = fin.tile([128, 512], F32, tag="bc")
                    nc.vector.reciprocal_approx_fast(out=bc_sb, in_=bc_ps)
                    ao_sb = aop.tile([128, 512], BF16, tag="ao")
                    nc.vector.tensor_mul(ao_sb, at_ps, bc_sb)
                    nc.sync.dma_start(
                        out=aT_qs[qi][h * 128:(h + 1) * 128, :],
                        in_=ao_sb,
                    )

        # ================= Phase 3: o_proj (partial) =================
        for qi in (3, 2, 1, 0):
            for tl in range(4):
                tt = qi * 4 + tl
                a_sb = ap3.tile([128, NH, 128], BF16, tag="a")
                nc.sync.dma_start(
                    out=a_sb,
                    in_=aT_qs[qi][:, tl * 128:(tl + 1) * 128].rearrange(
                        "(k p) t -> p k t", p=128
                    ),
                )
                for nh4 in range(4):     # output column blocks of 512
                    ps = pmm.tile([128, 512], F32, tag="pmm")
                    for k in range(NH):
                        nc.tensor.matmul(
                            ps,
                            a_sb[:, k, :],
                            ow_sb[:, k, nh4 * 512:(nh4 + 1) * 512],
                            start=(k == 0),
                            stop=(k == NH - 1),
                        )
                    o_sb = op3.tile([128, 512], F32, tag="o")
                    nc.scalar.copy(out=o_sb, in_=ps)
                    nc.sync.dma_start(
                        out=out_d[tt * 128:(tt + 1) * 128,
                                  nh4 * 512:(nh4 + 1) * 512],
                        in_=o_sb,
                    )

    nc.finalize()
    return nc


def _host_inputs(hidden_states, position_ids, qkv_w, qkv_b, o_w, has_bias):
    """Build the 8 per-core input maps (core c = 2*b + t)."""
    import ml_dtypes

    BF = ml_dtypes.bfloat16
    inv_freq = 1.0 / (
        ROPE_BASE ** (np.arange(0, HD, 2, dtype=np.float32) / HD)
    )
    ones = np.ones((128, 129), np.float32)

    # causal window: maskw[j, x] = 1 if (x - 384) >= j
    xg = np.arange(896)[None, :] - 384
    jg = np.arange(128)[:, None]
    maskw = (xg >= jg).astype(BF)

    per_t = []
    for t in range(TP):
        c0 = t * DOUT
        d = dict(
            wq=qkv_w[:, c0:c0 + DOUT].astype(BF),
            wk=qkv_w[:, H + c0:H + c0 + DOUT].astype(BF),
            wv=qkv_w[:, 2 * H + c0:2 * H + c0 + DOUT].astype(BF),
            ow=o_w[c0:c0 + DOUT, :].astype(BF),
        )
        if has_bias:
            d["bqkv"] = np.concatenate(
                [
                    qkv_b[c0:c0 + DOUT],
                    qkv_b[H + c0:H + c0 + DOUT],
                    qkv_b[2 * H + c0:2 * H + c0 + DOUT],
                ]
            ).reshape(1, 3 * DOUT).astype(BF)
            d["onesb"] = np.ones((1, 512), BF)
        per_t.append(d)

    in_maps = []
    for b in range(B):
        pos = position_ids[b].astype(np.float32)        # [S]
        freqs = pos[None, :] * inv_freq[:, None]        # [HD/2, S]
        emb = np.concatenate([freqs, freqs], axis=0)    # [HD, S]
        cosT = np.cos(emb).astype(np.float32)
        sinT = np.sin(emb).astype(np.float32)
        # pre-shuffled, sign-baked sin: t2_pre = q * sinTss, then swapping
        # partition halves of t2_pre yields rotate_half(q)*sin.
        sinTss = np.empty_like(sinT)
        sinTss[0:HD // 2] = sinT[HD // 2:HD]            # rows d<64: +sin[d+64]
        sinTss[HD // 2:HD] = -sinT[0:HD // 2]           # rows d>=64: -sin[d-64]
        xt = np.ascontiguousarray(hidden_states[b].T).astype(BF)
        for t in range(TP):
            in_maps.append(
                dict(
                    xt=xt,
                    cosT=cosT,
                    sinTss=sinTss,
                    maskw=maskw,
                    ones=ones,
                    onescb=np.ones((128, 1), BF),
                    **per_t[t],
                )
            )
    return in_maps


def _run(in_maps, has_bias, trace=False):
    from concourse.bass_utils import run_bass_kernel_spmd

    key = ("nc", has_bias)
    if key not in _CACHE:
        _CACHE[key] = _build(has_bias)
    return run_bass_kernel_spmd(
        _CACHE[key], in_maps, list(range(2 * B)), trace=trace
    )


def kernel(hidden_states, position_ids, qkv_w, qkv_b, o_w, _trace=False):
    hidden_states = np.asarray(hidden_states, dtype=np.float32)
    position_ids = np.asarray(position_ids)
    qkv_w = np.asarray(qkv_w, dtype=np.float32)
    qkv_b = np.asarray(qkv_b, dtype=np.float32)
    o_w = np.asarray(o_w, dtype=np.float32)

    has_bias = bool(np.any(qkv_b))
    in_maps = _host_inputs(
        hidden_states, position_ids, qkv_w, qkv_b, o_w, has_bias
    )
    res = _run(in_maps, has_bias, trace=_trace)
    _CACHE["last_results"] = res

    out = np.empty((B, S, H), dtype=np.float32)
    for b in range(B):
        out[b] = res.results[2 * b]["out"]
        out[b] += res.results[2 * b + 1]["out"]
    return out


# revision 18
# speedup vs baseline: 1.2434x; 1.0003x over previous
"""Causal self-attention block (qkv_proj + RoPE + SDPA + o_proj) on 8
Trainium2 NeuronCores.

Sharding: DP(batch)=4 x TP(heads)=2. Core c = 2*b + t handles batch b,
heads [8t, 8t+8). Each core computes its batch's qkv projection for its
heads, RoPE, causal attention, and a partial o_proj (Megatron row-shard);
the host sums the two partials per batch.

Matmul operands are bf16 (PE streams 1 cycle/row; fp32 PSUM accumulate).
The softmax-denominator path stays fp32r (12-bit mantissa fp32) so the
normalization is unbiased. End-to-end max rel err vs the fp32 reference
is ~4e-3 (dominated by bf16 operand rounding; fp32 accumulation
everywhere).

Layouts (per core):
  xt [2048 hid, 2048 tok] = X[b]^T host-transposed. Q/K projections emit
  qT/kT [1024 (head,d), 2048 tok] directly (lhsT = W tile, rhs = xt
  tile); RoPE applies in transposed layout (rotate-half = partition
  half-swap via SBUF->SBUF DMA). V emits natural [2048 tok, 1024]
  (lhsT = xt tile, rhs = Wv). Attention per head: scores^T [keys, q] =
  K^T-tile stationary @ q^T moving; exp on ACT (PSUM->SBUF, bf16 out);
  causal masking multiplies a sliding 0/1 window on diagonal blocks;
  fully-masked key blocks are skipped. attn^T [d, q] accumulates V-tile
  @ exp in PSUM. Softmax denominator: fp32r DVE-accumulated exp tiles,
  partition-reduced by a ones-column matmul, broadcast back via an
  outer-product matmul, wide reciprocal_approx_fast, one multiply.
  o_proj: out [tok, 2048] partial = attnT tiles stationary @ o_w rows.
"""
import sys

if "/opt/trn_rl_repo" not in sys.path:
    sys.path.insert(0, "/opt/trn_rl_repo")

import numpy as np

B = 4
S = 2048
H = 2048
NHEADS = 16
HD = 128
ROPE_BASE = 10000.0

TP = 2              # head-parallel ways
NH = NHEADS // TP   # heads per core = 8
DOUT = NH * HD      # q/k/v out dim per core = 1024
NKT = H // 128      # hidden contraction tiles = 16
SCALE = 1.0 / float(np.sqrt(HD))

_CACHE = {}


def _round_fp32r(x: np.ndarray) -> np.ndarray:
    bits = np.ascontiguousarray(x, dtype=np.float32).view(np.uint32)
    rnd = ((bits >> 12) & 1).astype(np.uint32)
    out = (bits + np.uint32(0x7FF) + rnd) & np.uint32(0xFFFFF000)
    return out.view(np.float32)


def _build(has_bias: bool):
    import concourse.tile as tile
    from concourse import bacc, mybir
    from contextlib import ExitStack

    F32 = mybir.dt.float32
    F32R = mybir.dt.float32r
    BF16 = mybir.dt.bfloat16
    EXP = mybir.ActivationFunctionType.Exp

    nc = bacc.Bacc(None)
    xt_d = nc.declare_dram_parameter("xt", [H, S], BF16, isOutput=False)
    wq_d = nc.declare_dram_parameter("wq", [H, DOUT], BF16, isOutput=False)
    wk_d = nc.declare_dram_parameter("wk", [H, DOUT], BF16, isOutput=False)
    wv_d = nc.declare_dram_parameter("wv", [H, DOUT], BF16, isOutput=False)
    ow_d = nc.declare_dram_parameter("ow", [DOUT, H], BF16, isOutput=False)
    cos_d = nc.declare_dram_parameter("cosT", [HD, S], F32, isOutput=False)
    sin_d = nc.declare_dram_parameter("sinTss", [HD, S], F32, isOutput=False)
    msk_d = nc.declare_dram_parameter("maskw", [128, 896], BF16, isOutput=False)
    one_d = nc.declare_dram_parameter("ones", [128, 129], F32R, isOutput=False)
    oneb_col_d = nc.declare_dram_parameter("onescb", [128, 1], BF16, isOutput=False)
    if has_bias:
        bqkv_d = nc.declare_dram_parameter(
            "bqkv", [1, 3 * DOUT], BF16, isOutput=False
        )
        oneb_d = nc.declare_dram_parameter("onesb", [1, 512], BF16, isOutput=False)
    out_d = nc.declare_dram_parameter("out", [S, H], F32, isOutput=True)

    with ExitStack() as ctx:
        tc = ctx.enter_context(tile.TileContext(nc))
        big = ctx.enter_context(tc.tile_pool(name="big", bufs=2))
        owp = ctx.enter_context(tc.tile_pool(name="owp", bufs=1))
        wst = ctx.enter_context(tc.tile_pool(name="wst", bufs=2))
        wvst = ctx.enter_context(tc.tile_pool(name="wvst", bufs=2))
        cst = ctx.enter_context(tc.tile_pool(name="cst", bufs=1))
        ev = ctx.enter_context(tc.tile_pool(name="ev", bufs=2))
        qst = ctx.enter_context(tc.tile_pool(name="qst", bufs=2))
        est = ctx.enter_context(tc.tile_pool(name="est", bufs=5))
        fin = ctx.enter_context(tc.tile_pool(name="fin", bufs=3))
        aop = ctx.enter_context(tc.tile_pool(name="ao", bufs=3))
        ap3 = ctx.enter_context(tc.tile_pool(name="ap3", bufs=2))
        op3 = ctx.enter_context(tc.tile_pool(name="op3", bufs=3))
        pmm = ctx.enter_context(tc.tile_pool(name="pmm", bufs=4, space="PSUM"))
        patt = ctx.enter_context(tc.tile_pool(name="patt", bufs=2, space="PSUM"))
        prs = ctx.enter_context(tc.tile_pool(name="prs", bufs=2, space="PSUM"))
        dram = ctx.enter_context(tc.tile_pool(name="dram", bufs=1, space="DRAM"))

        # ---- DRAM scratch (tile-tracked for cross-phase deps) ----
        qT_s = dram.tile([DOUT, S], BF16, tag="qT")
        kT_s = dram.tile([DOUT, S], BF16, tag="kT")
        v_s = dram.tile([S, DOUT], BF16, tag="v")
        # per-q-block attn^T scratch so o_proj(qi) only waits on its block
        aT_qs = [
            dram.tile([DOUT, 512], BF16, tag=f"aT{qi}", name=f"aT{qi}")
            for qi in range(4)
        ]

        # ---- constants ----
        ones_sb = cst.tile([128, 129], F32R, tag="ones")
        nc.sync.dma_start(out=ones_sb, in_=one_d[:, :])
        ones_row = ones_sb[0:1, 0:128]          # [1, 128] fp32r
        onescb_sb = cst.tile([128, 1], BF16, tag="onescb")
        nc.sync.dma_start(out=onescb_sb, in_=oneb_col_d[:, :])
        cos_sb = cst.tile([HD, S], F32, tag="cos")
        nc.sync.dma_start(out=cos_sb, in_=cos_d[:, :])
        sin_sb = cst.tile([HD, S], F32, tag="sin")
        nc.sync.dma_start(out=sin_sb, in_=sin_d[:, :])
        msk_sb = cst.tile([128, 896], BF16, tag="msk")
        nc.sync.dma_start(out=msk_sb, in_=msk_d[:, :])
        if has_bias:
            bqkv_sb = cst.tile([1, 3 * DOUT], BF16, tag="bqkv")
            nc.sync.dma_start(out=bqkv_sb, in_=bqkv_d[:, :])
            onesb_sb = cst.tile([1, 512], BF16, tag="onesb")
            nc.sync.dma_start(out=onesb_sb, in_=oneb_d[:, :])

        # ================= Phase 1: QKV projection =================
        for th in range(2):          # token halves of 1024
            t0 = th * 1024
            xt_t = big.tile([128, NKT, 1024], BF16, tag="big")
            for k in range(NKT):
                nc.sync.dma_start(
                    out=xt_t[:, k, :],
                    in_=xt_d[k * 128:(k + 1) * 128, t0:t0 + 1024],
                )

            # --- Q and K: transposed output + RoPE ---
            for wi, (w_d, out_s) in enumerate(((wq_d, qT_s), (wk_d, kT_s))):
                for dt in range(NH):
                    w_t = wst.tile([128, NKT, 128], BF16, tag="w")
                    nc.sync.dma_start(
                        out=w_t,
                        in_=w_d[:, dt * 128:(dt + 1) * 128].rearrange(
                            "(k p) m -> p k m", p=128
                        ),
                    )
                    for tq in range(2):
                        q0 = t0 + tq * 512
                        ps = pmm.tile([128, 512], F32, tag="pmm")
                        if has_bias:
                            bias = bqkv_sb[:, wi * DOUT + dt * 128:
                                           wi * DOUT + (dt + 1) * 128]
                            nc.tensor.matmul(
                                ps, bias, onesb_sb, start=True, stop=False
                            )
                        for k in range(NKT):
                            nc.tensor.matmul(
                                ps,
                                w_t[:, k, :],
                                xt_t[:, k, tq * 512:(tq + 1) * 512],
                                start=(k == 0 and not has_bias),
                                stop=(k == NKT - 1),
                            )
                        # RoPE: q' = q*cos + halfswap(q*sin_shuf)
                        # (sinTss is the host-preshuffled, sign-baked sin)
                        qs = ev.tile([128, 512], F32, tag="qs")
                        nc.vector.tensor_mul(qs, ps, cos_sb[:, q0:q0 + 512])
                        t2p = ev.tile([128, 512], F32, tag="t2p")
                        nc.vector.tensor_mul(t2p, ps, sin_sb[:, q0:q0 + 512])
                        qr = ev.tile([128, 512], F32, tag="qr")
                        nc.sync.dma_start(out=qr[0:64, :], in_=t2p[64:128, :])
                        nc.sync.dma_start(out=qr[64:128, :], in_=t2p[0:64, :])
                        qo = ev.tile([128, 512], BF16, tag="qo")
                        nc.vector.tensor_add(qo, qs, qr)
                        nc.sync.dma_start(
                            out=out_s[dt * 128:(dt + 1) * 128, q0:q0 + 512],
                            in_=qo,
                        )

            # --- V: natural output ---
            for dv in range(2):      # dout chunks of 512
                wv_t = wvst.tile([128, NKT, 512], BF16, tag="wv")
                nc.sync.dma_start(
                    out=wv_t,
                    in_=wv_d[:, dv * 512:(dv + 1) * 512].rearrange(
                        "(k p) m -> p k m", p=128
                    ),
                )
                for tt in range(8):  # token tiles of 128 in this half
                    ps = pmm.tile([128, 512], F32, tag="pmm")
                    if has_bias:
                        bias_v = bqkv_sb[:, 2 * DOUT + dv * 512:
                                         2 * DOUT + (dv + 1) * 512]
                        nc.tensor.matmul(
                            ps, onesb_sb[:, 0:128], bias_v,
                            start=True, stop=False,
                        )
                    for k in range(NKT):
                        nc.tensor.matmul(
                            ps,
                            xt_t[:, k, tt * 128:(tt + 1) * 128],
                            wv_t[:, k, :],
                            start=(k == 0 and not has_bias),
                            stop=(k == NKT - 1),
                        )
                    vo = ev.tile([128, 512], BF16, tag="qo")
                    nc.scalar.copy(out=vo, in_=ps)
                    nc.sync.dma_start(
                        out=v_s[t0 + tt * 128:t0 + (tt + 1) * 128,
                                dv * 512:(dv + 1) * 512],
                        in_=vo,
                    )

        # ================= Phase 2: causal attention =================
        ow_sb = owp.tile([128, NH, H], BF16, tag="ow")
        nc.sync.dma_start(
            out=ow_sb, in_=ow_d.rearrange("(k p) n -> p k n", p=128)
        )
        for hp in range(NH // 2):    # head pairs
            kv = big.tile([128, 2, 2, NKT, 128], BF16, tag="big")
            for hh in range(2):
                h = hp * 2 + hh
                nc.sync.dma_start(
                    out=kv[:, hh, 0, :, :],
                    in_=kT_s[h * 128:(h + 1) * 128, :].rearrange(
                        "p (kt t) -> p kt t", t=128
                    ),
                )
                nc.sync.dma_start(
                    out=kv[:, hh, 1, :, :],
                    in_=v_s[:, h * 128:(h + 1) * 128].rearrange(
                        "(kt p) d -> p kt d", p=128
                    ),
                )
            for hh in range(2):
                h = hp * 2 + hh
                q_sb = qst.tile([128, S], BF16, tag="q")
                nc.sync.dma_start(
                    out=q_sb, in_=qT_s[h * 128:(h + 1) * 128, :]
                )
                for qi in (3, 2, 1, 0):
                    nkj = 4 * (qi + 1)
                    qsl = q_sb[:, qi * 512:(qi + 1) * 512]
                    at_ps = patt.tile([128, 512], F32, tag="patt")
                    rs_ps = prs.tile([1, 512], F32, tag="prs")
                    for kj in range(nkj):
                        s_ps = pmm.tile([128, 512], F32, tag="pmm")
                        nc.tensor.matmul(
                            s_ps, kv[:, hh, 0, kj, :], qsl,
                            start=True, stop=True,
                        )
                        e_sb = est.tile([128, 512], BF16, tag="e")
                        nc.scalar.activation(
                            out=e_sb, in_=s_ps, func=EXP, scale=SCALE
                        )
                        m = kj - (nkj - 4)
                        if m >= 0:   # diagonal block: causal window
                            nc.vector.tensor_mul(
                                e_sb, e_sb,
                                msk_sb[:, (3 - m) * 128:(3 - m) * 128 + 512],
                            )
                        nc.tensor.matmul(
                            at_ps, kv[:, hh, 1, kj, :], e_sb,
                            start=(kj == 0), stop=(kj == nkj - 1),
                        )
                        # softmax denominator accumulates on the PE
                        nc.tensor.matmul(
                            rs_ps, onescb_sb, e_sb,
                            start=(kj == 0), stop=(kj == nkj - 1),
                        )
                    # broadcast denominator, reciprocal, normalize
                    rs_sb = fin.tile([1, 512], F32R, tag="r")
                    nc.scalar.copy(out=rs_sb, in_=rs_ps)
                    bc_ps = pmm.tile([128, 512], F32, tag="pmm")
                    nc.tensor.matmul(
                        bc_ps, ones_row, rs_sb, start=True, stop=True
                    )
                    bc_sb = fin.tile([128, 512], F32, tag="bc")
                    nc.vector.reciprocal_approx_fast(out=bc_sb, in_=bc_ps)
                    ao_sb = aop.tile([128, 512], BF16, tag="ao")
                    nc.vector.tensor_mul(ao_sb, at_ps, bc_sb)
                    nc.sync.dma_start(
                        out=aT_qs[qi][h * 128:(h + 1) * 128, :],
                        in_=ao_sb,
                    )

        # ================= Phase 3: o_proj (partial) =================
        for qi in (3, 2, 1, 0):
            for tl in range(4):
                tt = qi * 4 + tl
                a_sb = ap3.tile([128, NH, 128], BF16, tag="a")
                nc.sync.dma_start(
                    out=a_sb,
                    in_=aT_qs[qi][:, tl * 128:(tl + 1) * 128].rearrange(
                        "(k p) t -> p k t", p=128
                    ),
                )
                for nh4 in range(4):     # output column blocks of 512
                    ps = pmm.tile([128, 512], F32, tag="pmm")
                    for k in range(NH):
                        nc.tensor.matmul(
                            ps,
                            a_sb[:, k, :],
                            ow_sb[:, k, nh4 * 512:(nh4 + 1) * 512],
                            start=(k == 0),
                            stop=(k == NH - 1),
                        )
                    o_sb = op3.tile([128, 512], F32, tag="o")
                    nc.scalar.copy(out=o_sb, in_=ps)
                    nc.sync.dma_start(
                        out=out_d[tt * 128:(tt + 1) * 128,
                                  nh4 * 512:(nh4 + 1) * 512],
                        in_=o_sb,
                    )

    nc.finalize()
    return nc


def _host_inputs(hidden_states, position_ids, qkv_w, qkv_b, o_w, has_bias):
    """Build the 8 per-core input maps (core c = 2*b + t)."""
    import ml_dtypes

    BF = ml_dtypes.bfloat16
    inv_freq = 1.0 / (
        ROPE_BASE ** (np.arange(0, HD, 2, dtype=np.float32) / HD)
    )
    ones = np.ones((128, 129), np.float32)

    # causal window: maskw[j, x] = 1 if (x - 384) >= j
    xg = np.arange(896)[None, :] - 384
    jg = np.arange(128)[:, None]
    maskw = (xg >= jg).astype(BF)

    per_t = []
    for t in range(TP):
        c0 = t * DOUT
        d = dict(
            wq=qkv_w[:, c0:c0 + DOUT].astype(BF),
            wk=qkv_w[:, H + c0:H + c0 + DOUT].astype(BF),
            wv=qkv_w[:, 2 * H + c0:2 * H + c0 + DOUT].astype(BF),
            ow=o_w[c0:c0 + DOUT, :].astype(BF),
        )
        if has_bias:
            d["bqkv"] = np.concatenate(
                [
                    qkv_b[c0:c0 + DOUT],
                    qkv_b[H + c0:H + c0 + DOUT],
                    qkv_b[2 * H + c0:2 * H + c0 + DOUT],
                ]
            ).reshape(1, 3 * DOUT).astype(BF)
            d["onesb"] = np.ones((1, 512), BF)
        per_t.append(d)

    in_maps = []
    for b in range(B):
        pos = position_ids[b].astype(np.float32)        # [S]
        freqs = pos[None, :] * inv_freq[:, None]        # [HD/2, S]
        emb = np.concatenate([freqs, freqs], axis=0)    # [HD, S]
        cosT = np.cos(emb).astype(np.float32)
        sinT = np.sin(emb).astype(np.float32)
        # pre-shuffled, sign-baked sin: t2_pre = q * sinTss, then swapping
        # partition halves of t2_pre yields rotate_half(q)*sin.
        sinTss = np.empty_like(sinT)
        sinTss[0:HD // 2] = sinT[HD // 2:HD]            # rows d<64: +sin[d+64]
        sinTss[HD // 2:HD] = -sinT[0:HD // 2]           # rows d>=64: -sin[d-64]
        xt = np.ascontiguousarray(hidden_states[b].T).astype(BF)
        for t in range(TP):
            in_maps.append(
                dict(
                    xt=xt,
                    cosT=cosT,
                    sinTss=sinTss,
                    maskw=maskw,
                    ones=ones,
                    onescb=np.ones((128, 1), BF),
                    **per_t[t],
                )
            )
    return in_maps


def _run(in_maps, has_bias, trace=False):
    from concourse.bass_utils import run_bass_kernel_spmd

    key = ("nc", has_bias)
    if key not in _CACHE:
        _CACHE[key] = _build(has_bias)
    return run_bass_kernel_spmd(
        _CACHE[key], in_maps, list(range(2 * B)), trace=trace
    )


def kernel(hidden_states, position_ids, qkv_w, qkv_b, o_w, _trace=False):
    hidden_states = np.asarray(hidden_states, dtype=np.float32)
    position_ids = np.asarray(position_ids)
    qkv_w = np.asarray(qkv_w, dtype=np.float32)
    qkv_b = np.asarray(qkv_b, dtype=np.float32)
    o_w = np.asarray(o_w, dtype=np.float32)

    has_bias = bool(np.any(qkv_b))
    in_maps = _host_inputs(
        hidden_states, position_ids, qkv_w, qkv_b, o_w, has_bias
    )
    res = _run(in_maps, has_bias, trace=_trace)
    _CACHE["last_results"] = res

    out = np.empty((B, S, H), dtype=np.float32)
    for b in range(B):
        out[b] = res.results[2 * b]["out"]
        out[b] += res.results[2 * b + 1]["out"]
    return out
